# revision 23
# baseline (speedup 1.0000x reference)
"""Causal self-attention (dense transformer attn layer) on 8 Trainium2
NeuronCores.

Sharding: batch x head-group.  Core c handles batch b = c//2 and head-group
g = c%2 (8 of 16 heads).  Each core computes the qkv projection for its head
slice (column-parallel), full causal attention for its 8 heads, and a
row-parallel slice of the output projection.  The host sums the two partial
projection outputs per batch (the "all-reduce") and adds b_proj.

On-core DRAM layout (per core, T=2048, C=1024, HLOC=8 heads, D=64):
  xT   [C, T]    input slice, transposed on host      (bf16)
  wqk  [C, 1024] W_attn columns for q (512) + k (512) (bf16)
  wv   [C, 512]  W_attn columns for v                 (bf16)
  wpr  [512, C]  W_proj rows for this group           (bf16)
  bqk  [128, 8]  q/k bias per dout-chunk column       (f32)
  bv   [1, 512]  v bias                               (bf16)
  out  [T, C]    partial output                       (f32)

All big matmuls run in bf16 (1 PE cycle/row) with fp32 PSUM accumulation.
The kernel is software-pipelined per 512-token i-chunk: while chunk c4's
attention runs, chunk c4+1's qkv-projection matmuls are interleaved into
the PE stream so the tensor engine stays dense (HAM stays at K=8/8).

Per i-chunk:
  1. q(i)^T = wq-stationary @ x(i)^T    -> [d, t] layout
  2. k(i)^T = wk-stationary @ x(i)^T    -> per-chunk k^T tile
  3. v(i)   = x(i)^T-stationary @ wv    -> [t, d] + ones column (for l)
  4. per head, per causal j-block (128 keys):
       S^T[j, i] = k^T(j)-stationary @ q^T(i)     (contract d=64, row-pair
                                                   packed via tile_position)
       P^T       = exp(S^T/sqrt(d)) on the causal region; above-diagonal
                   region zeroed / masked multiplicatively (bf16 DVE)
       Yaug^T   += [V|1](j)-stationary @ P^T      (contract j=128, accum)
     Yaug^T row 64 is the softmax denominator l(i);
     1/l = exp(-ln(l)) on ScalarE, broadcast to 64 partitions via a
     rank-1 PE matmul, y^T = Y^T * bcast(1/l) on VectorE
  5. out(i) = y^T-stationary @ wpr                (contract d=512, accum 4)
"""

import numpy as np

# ---------------------------------------------------------------- constants
B, T, C = 4, 2048, 1024
H, D = 16, 64
NCORES = 8
HGROUPS = NCORES // B          # 2 head groups
HLOC = H // HGROUPS            # 8 heads per core
DQ = HLOC * D                  # 512 head-dims per core
P = 128
IC = 512                       # i-chunk (query) width


def _import_concourse():
    try:
        import concourse.bass  # noqa: F401
    except ImportError:
        import sys

        for p in ("/opt/trn_rl_repo", "/root/.axon_site/_ro/trn_rl_repo"):
            if p not in sys.path:
                sys.path.insert(0, p)
        import concourse.bass  # noqa: F401


def build_program(t=T, c=C, hloc=HLOC, d=D):
    """Build the single-core Bass program (the same program runs SPMD on 8)."""
    _import_concourse()
    import concourse.bass as bass
    import concourse.mybir as mybir
    import concourse.tile as tile

    assert c % P == 0 and t % IC == 0 and hloc % 2 == 0 and d == 64
    dq = hloc * d                  # local q/k/v width
    CK = c // P                    # contraction chunks over channels
    TI = t // IC                   # i-chunks
    JPC = IC // P                  # j-blocks per i-chunk (4)
    DCH = dq // P                  # q/k/y dout chunks
    HP = hloc // 2                 # head pairs
    F32 = mybir.dt.float32
    R32 = mybir.dt.float32r
    BF16 = mybir.dt.bfloat16
    EXP = mybir.ActivationFunctionType.Exp
    LN = mybir.ActivationFunctionType.Ln
    SCALE = 1.0 / float(np.sqrt(d))

    nc = bass.Bass()
    xT = nc.declare_dram_parameter("xT", [c, t], BF16, isOutput=False)
    wqk = nc.declare_dram_parameter("wqk", [c, 2 * dq], BF16, isOutput=False)
    wv = nc.declare_dram_parameter("wv", [c, dq], BF16, isOutput=False)
    wpr = nc.declare_dram_parameter("wpr", [dq, c], BF16, isOutput=False)
    bqk = nc.declare_dram_parameter("bqk", [P, 2 * DCH], F32, isOutput=False)
    bv = nc.declare_dram_parameter("bv", [1, dq], BF16, isOutput=False)
    out = nc.declare_dram_parameter("out", [t, c], F32, isOutput=True)

    with tile.TileContext(nc) as tc:
        with (
            # float32r is fp32-width; the low-precision guard is dtype-strict
            nc.allow_low_precision(reason="bf16 matmul inputs, fp32 accum"),
            tc.tile_pool(name="const", bufs=1) as const,
            tc.tile_pool(name="xin", bufs=18) as xin,
            tc.tile_pool(name="qpool", bufs=2) as qpool,
            tc.tile_pool(name="kpool", bufs=TI) as kpool,
            tc.tile_pool(name="vpool", bufs=TI) as vpool,
            tc.tile_pool(name="ypool", bufs=2) as ypool,
            tc.tile_pool(name="ptp", bufs=6) as ptp,
            tc.tile_pool(name="bcp", bufs=2) as bcp,
            tc.tile_pool(name="ytp", bufs=2) as ytp,
            tc.tile_pool(name="ostage", bufs=2) as ostage,
            tc.tile_pool(name="ps_mm", bufs=2, space="PSUM") as ps_mm,
            tc.tile_pool(name="ps_st", bufs=3, space="PSUM") as ps_st,
            tc.tile_pool(name="ps_y", bufs=3, space="PSUM") as ps_y,
        ):
            # ---------------- persistent SBUF state
            wqk_sb = const.tile([P, CK, 2 * dq], BF16)
            wv_sb = const.tile([P, CK, dq], BF16)
            wpr_sb = const.tile([P, DCH, c], BF16)
            mask_sb = const.tile([P, JPC, IC], BF16)
            ones_sb = const.tile([P, P], R32)
            ones_bf = const.tile([P, P], BF16)
            bqk_sb = const.tile([P, 2 * DCH], F32)
            bv_sb = const.tile([1, dq], BF16)

            for cc in range(CK):
                nc.sync.dma_start(out=wqk_sb[:, cc, :], in_=wqk[cc * P:(cc + 1) * P, :])
                nc.sync.dma_start(out=wv_sb[:, cc, :], in_=wv[cc * P:(cc + 1) * P, :])
            for dc in range(DCH):
                nc.sync.dma_start(out=wpr_sb[:, dc, :], in_=wpr[dc * P:(dc + 1) * P, :])
            nc.sync.dma_start(out=bqk_sb, in_=bqk[:, :])
            nc.sync.dma_start(out=bv_sb, in_=bv[:, :])

            # memset can't emit float32r directly (invalid ISA); fill an f32
            # scratch and round it via DVE copies
            ones_f32 = const.tile([P, P], F32)
            nc.vector.memset(ones_f32, 1.0)
            nc.vector.tensor_copy(out=ones_sb, in_=ones_f32)
            nc.vector.tensor_copy(out=ones_bf, in_=ones_f32)
            # multiplicative causal masks for the 4 diagonal j-block
            # positions: pattern p is 1 where i_local >= j_local + 128*p
            for pat in range(JPC):
                nc.gpsimd.memset(mask_sb[:, pat, :], 1.0)
                nc.gpsimd.affine_select(
                    out=mask_sb[:, pat, :],
                    in_=mask_sb[:, pat, :],
                    compare_op=mybir.AluOpType.is_ge,
                    fill=0.0,
                    base=-(pat * P),
                    pattern=[[1, IC]],
                    channel_multiplier=-1,
                )

            q_tiles = {}
            k_tiles = {}
            v_tiles = {}

            def load_x(c4):
                isl = slice(c4 * IC, (c4 + 1) * IC)
                xt = []
                for cc in range(CK):
                    xtile = xin.tile([P, IC], BF16, tag="x")
                    nc.sync.dma_start(out=xtile,
                                      in_=xT[cc * P:(cc + 1) * P, isl])
                    xt.append(xtile)
                return xt

            def qkv_thunks(c4, xt):
                """One thunk per PSUM accumulation group; called interleaved
                with the previous chunk's attention to keep PE dense."""
                q_cur = qpool.tile([P, DCH, IC], BF16, tag="q")
                k_cur = kpool.tile([P, DCH, IC], BF16, tag="k")
                v_cur = vpool.tile([P, JPC, hloc, d + 1], BF16, tag="v")
                q_tiles[c4] = q_cur
                k_tiles[c4] = k_cur
                v_tiles[c4] = v_cur
                thunks = []

                def q_group(oc):
                    ps = ps_mm.tile([P, 512], F32, tag="mm")
                    for cc in range(CK):
                        nc.tensor.matmul(
                            ps[:, :IC],
                            lhsT=wqk_sb[:, cc, oc * P:(oc + 1) * P],
                            rhs=xt[cc], start=(cc == 0), stop=(cc == CK - 1))
                    nc.vector.tensor_scalar_add(q_cur[:, oc, :], ps[:, :IC],
                                                bqk_sb[:, oc:oc + 1])

                def k_group(oc):
                    ps = ps_mm.tile([P, 512], F32, tag="mm")
                    for cc in range(CK):
                        nc.tensor.matmul(
                            ps[:, :IC],
                            lhsT=wqk_sb[:, cc, dq + oc * P:dq + (oc + 1) * P],
                            rhs=xt[cc], start=(cc == 0), stop=(cc == CK - 1))
                    nc.vector.tensor_scalar_add(
                        k_cur[:, oc, :], ps[:, :IC],
                        bqk_sb[:, DCH + oc:DCH + oc + 1])

                def v_group(tbl):
                    ps = ps_mm.tile([P, 512], F32, tag="mm")
                    for cc in range(CK):
                        nc.tensor.matmul(
                            ps[:, :dq],
                            lhsT=xt[cc][:, tbl * P:(tbl + 1) * P],
                            rhs=wv_sb[:, cc, :], start=(cc == 0), stop=False)
                    nc.tensor.matmul(ps[:, :dq], lhsT=ones_bf[0:1, :],
                                     rhs=bv_sb[0:1, :], start=False, stop=True)
                    nc.vector.tensor_copy(
                        out=v_cur[:, tbl, :, 0:d],
                        in_=ps[:, :dq].rearrange("p (h e) -> p h e", h=hloc))
                    # ones column for the softmax-denominator accumulator
                    nc.vector.tensor_copy(
                        out=v_cur[:, tbl, :, d:d + 1],
                        in_=ones_bf[:, 0:hloc][:, :, None])

                for oc in range(DCH):
                    thunks.append(lambda oc=oc: q_group(oc))
                    thunks.append(lambda oc=oc: k_group(oc))
                for tbl in range(JPC):
                    thunks.append(lambda tbl=tbl: v_group(tbl))
                return thunks

            def attention_hp(c4, hp, filler=()):
                filler = list(filler)
                q_cur = q_tiles[c4]
                njb = (c4 + 1) * JPC
                fill_every = max(1, njb // len(filler)) if filler else 0
                ya = ps_y.tile([d + 1, IC], F32, tag="y")
                yb = ps_y.tile([d + 1, IC], F32, tag="y")
                for jb in range(njb):
                    kc, jl = jb // JPC, jb % JPC
                    for hi, po, yps in ((0, 0, ya), (1, 64, yb)):
                        h = 2 * hp + hi
                        st = ps_st.tile([P, IC], F32, tag="st")
                        pt = ptp.tile([P, IC], BF16, tag="pt")
                        if jb >= c4 * JPC:
                            # diagonal: columns < w0 fully masked (zeroed on
                            # the otherwise-idle GpSimd engine), triangular
                            # 128-col block masked multiplicatively on DVE
                            pat = jb - c4 * JPC
                            w0 = pat * P
                            nc.tensor.matmul(
                                st[:, w0:],
                                lhsT=k_tiles[kc][po:po + 64, hp,
                                                 jl * P:(jl + 1) * P],
                                rhs=q_cur[po:po + 64, hp, w0:],
                                start=True, stop=True)
                            nc.scalar.activation(pt[:, w0:], st[:, w0:], EXP,
                                                 scale=SCALE)
                            if w0:
                                nc.gpsimd.memset(pt[:, :w0], 0.0)
                            nc.vector.tensor_mul(
                                pt[:, w0:w0 + P], pt[:, w0:w0 + P],
                                mask_sb[:, pat, w0:w0 + P])
                        else:
                            nc.tensor.matmul(
                                st,
                                lhsT=k_tiles[kc][po:po + 64, hp,
                                                 jl * P:(jl + 1) * P],
                                rhs=q_cur[po:po + 64, hp, :],
                                start=True, stop=True)
                            nc.scalar.activation(pt, st, EXP, scale=SCALE)
                        nc.tensor.matmul(
                            yps, lhsT=v_tiles[jb // JPC][:, jb % JPC, h, :],
                            rhs=pt, start=(jb == 0), stop=(jb == njb - 1))
                    if filler and (jb + 1) % fill_every == 0:
                        filler.pop(0)()
                for th in filler:
                    th()
                # normalize: y^T[e, i] = Y^T[e, i] * (1/l[i]); 1/l computed
                # as exp(-ln(l)) on ScalarE (DVE reciprocal on [1,512] is
                # 3.3us), then partition-broadcast via a rank-1 PE matmul.
                y_cur = y_tiles[c4]
                for hi, po, yps in ((0, 0, ya), (1, 64, yb)):
                    lrow = bcp.tile([P, IC], F32, tag="lrow")
                    nc.scalar.activation(lrow[d:d + 1, :], yps[d:d + 1, :], LN)
                    rinv = bcp.tile([P, IC], R32, tag="rinv")
                    nc.scalar.activation(rinv[d:d + 1, :], lrow[d:d + 1, :],
                                         EXP, scale=-1.0)
                    bc = ps_y.tile([d + 1, IC], F32, tag="y")
                    nc.tensor.matmul(
                        bc[0:d, :], lhsT=ones_sb[d:d + 1, 0:d],
                        rhs=rinv[d:d + 1, :], start=True, stop=True)
                    bcs = bcp.tile([P, IC], F32, tag="bcs")
                    nc.vector.tensor_copy(out=bcs[0:d, :], in_=bc[0:d, :])
                    if hi == 0:
                        nc.vector.tensor_mul(y_cur[0:d, hp, :],
                                             yps[0:d, :], bcs[0:d, :])
                    else:
                        yt = ytp.tile([P, IC], BF16, tag="yt")
                        nc.vector.tensor_mul(yt[0:d, :],
                                             yps[0:d, :], bcs[0:d, :])
                        # shift to partitions 64..127 (SBUF->SBUF DMA)
                        nc.sync.dma_start(out=y_cur[64:P, hp, :],
                                          in_=yt[0:d, :])

            def proj_thunks(c4):
                y_cur = y_tiles[c4]

                def grp(tbl, oh):
                    tb = c4 * JPC + tbl
                    ps = ps_mm.tile([P, 512], F32, tag="mm")
                    for dc in range(DCH):
                        nc.tensor.matmul(
                            ps,
                            lhsT=y_cur[:, dc, tbl * P:(tbl + 1) * P],
                            rhs=wpr_sb[:, dc, oh * 512:(oh + 1) * 512],
                            start=(dc == 0), stop=(dc == DCH - 1))
                    ost = ostage.tile([P, 512], F32, tag="ost")
                    nc.vector.tensor_copy(out=ost, in_=ps)
                    nc.sync.dma_start(
                        out=out[tb * P:(tb + 1) * P,
                                oh * 512:(oh + 1) * 512],
                        in_=ost)

                return [lambda tbl=tbl, oh=oh: grp(tbl, oh)
                        for tbl in range(JPC) for oh in range(c // 512)]

            # -------------- software pipeline over i-chunks
            y_tiles = {}
            xt = load_x(0)
            for th in qkv_thunks(0, xt):
                th()
            prev_proj = []
            for c4 in range(TI):
                pend = list(prev_proj)
                if c4 + 1 < TI:
                    xt = load_x(c4 + 1)
                    pend += qkv_thunks(c4 + 1, xt)
                y_tiles[c4] = ypool.tile([P, DCH, IC], BF16, tag="ych",
                                         name=f"ych_{c4}")
                per_hp = (len(pend) + HP - 1) // HP if pend else 0
                for hp in range(HP):
                    attention_hp(c4, hp,
                                 filler=pend[hp * per_hp:(hp + 1) * per_hp])
                prev_proj = proj_thunks(c4)
            for th in prev_proj:
                th()

    _split_multi_waits(nc, mybir)
    return nc


def _split_multi_waits(nc, mybir):
    """The walrus build in this image rejects instructions carrying more than
    one sem wait ("Too many sync wait commands").  Tile's exit drain carries
    several; peel the extras onto same-engine nops placed just before."""
    for f in nc.m.functions:
        for blk in f.blocks:
            changed = False
            out_list = []
            for inst in blk.instructions:
                si = inst.sync_info
                if si is not None and len(si.on_wait) > 1:
                    waits = list(si.on_wait)
                    for j, w in enumerate(waits[1:]):
                        nop = mybir.InstNoOp(
                            name=f"{inst.name}-wsplit-{j}", ins=[], outs=[],
                            sync_info=mybir.SyncInfo(on_update=[], on_wait=[w]))
                        nop.engine = inst.engine
                        try:
                            nc.register_instruction(nop, overwrite=True)
                        except Exception:
                            pass
                        out_list.append(nop)
                    si.on_wait = waits[:1]
                    inst.sync_info = si
                    changed = True
                out_list.append(inst)
            if changed:
                blk.instructions = out_list


# ------------------------------------------------------------------- host
_cache = {}


def _get_program():
    if "nc" not in _cache:
        _cache["nc"] = build_program()
    return _cache["nc"]


def make_in_maps(x, W_attn, b_attn, W_proj, b_proj):
    import ml_dtypes

    bf16 = ml_dtypes.bfloat16
    x = np.asarray(x, np.float32)
    W_attn = np.asarray(W_attn, np.float32)
    b_attn = np.asarray(b_attn, np.float32)
    W_proj = np.asarray(W_proj, np.float32)
    in_maps = []
    for core in range(NCORES):
        b = core // HGROUPS
        g = core % HGROUPS
        hs = g * DQ
        wq = W_attn[:, hs:hs + DQ]
        wk = W_attn[:, C + hs:C + hs + DQ]
        wv = W_attn[:, 2 * C + hs:2 * C + hs + DQ]
        bq = b_attn[hs:hs + DQ]
        bk = b_attn[C + hs:C + hs + DQ]
        bv = b_attn[2 * C + hs:2 * C + hs + DQ]
        in_maps.append({
            "xT": np.ascontiguousarray(x[b].T).astype(bf16),
            "wqk": np.concatenate([wq, wk], axis=1).astype(bf16),
            "wv": np.ascontiguousarray(wv).astype(bf16),
            "wpr": np.ascontiguousarray(W_proj[hs:hs + DQ, :]).astype(bf16),
            "bqk": np.ascontiguousarray(
                np.concatenate([bq, bk]).reshape(2 * (DQ // P), P).T),
            "bv": bv.reshape(1, DQ).astype(bf16),
        })
    return in_maps


def combine_outputs(outs, b_proj):
    b_proj = np.asarray(b_proj, np.float32)
    y = np.empty((B, T, C), np.float32)
    for b in range(B):
        y[b] = outs[HGROUPS * b] + outs[HGROUPS * b + 1]
    y += b_proj[None, None, :]
    return y


def kernel(x, W_attn, b_attn, W_proj, b_proj):
    _import_concourse()
    from concourse.bass_utils import run_bass_kernel_spmd

    nc = _get_program()
    in_maps = make_in_maps(x, W_attn, b_attn, W_proj, b_proj)
    res = run_bass_kernel_spmd(nc, in_maps, core_ids=list(range(NCORES)))
    outs = [res.results[i]["out"] for i in range(NCORES)]
    return combine_outputs(outs, b_proj)


# revision 24
# speedup vs baseline: 1.0516x; 1.0516x over previous
"""Causal self-attention (dense transformer attn layer) on 8 Trainium2
NeuronCores.

Sharding: batch x head-group.  Core c handles batch b = c//2 and head-group
g = c%2 (8 of 16 heads).  Each core computes the qkv projection for its head
slice (column-parallel), full causal attention for its 8 heads, and a
row-parallel slice of the output projection.  The host sums the two partial
projection outputs per batch (the "all-reduce") and adds b_proj.

On-core DRAM layout (per core, T=2048, C=1024, HLOC=8 heads, D=64):
  xT   [C, T]    input slice, transposed on host      (bf16)
  wqk  [C, 1024] W_attn columns for q (512) + k (512) (bf16)
  wv   [C, 512]  W_attn columns for v                 (bf16)
  wpr  [512, C]  W_proj rows for this group           (bf16)
  bqk  [128, 8]  q/k bias per dout-chunk column       (f32)
  bv   [1, 512]  v bias                               (bf16)
  out  [T, C]    partial output                       (f32)

All big matmuls run in bf16 (1 PE cycle/row) with fp32 PSUM accumulation.
The kernel is software-pipelined per 512-token i-chunk: while chunk c4's
attention runs, chunk c4+1's qkv-projection matmuls are interleaved into
the PE stream so the tensor engine stays dense (HAM stays at K=8/8).

Per i-chunk:
  1. q(i)^T = wq-stationary @ x(i)^T    -> [d, t] layout
  2. k(i)^T = wk-stationary @ x(i)^T    -> per-chunk k^T tile
  3. v(i)   = x(i)^T-stationary @ wv    -> [t, d] + ones column (for l)
  4. per head, per causal j-block (128 keys):
       S^T[j, i] = k^T(j)-stationary @ q^T(i)     (contract d=64, row-pair
                                                   packed via tile_position)
       P^T       = exp(S^T/sqrt(d)) on the causal region; above-diagonal
                   region zeroed / masked multiplicatively (bf16 DVE)
       Yaug^T   += [V|1](j)-stationary @ P^T      (contract j=128, accum)
     Yaug^T row 64 is the softmax denominator l(i);
     1/l = exp(-ln(l)) on ScalarE, broadcast to 64 partitions via a
     rank-1 PE matmul, y^T = Y^T * bcast(1/l) on VectorE
  5. out(i) = y^T-stationary @ wpr                (contract d=512, accum 4)
"""

import numpy as np

# ---------------------------------------------------------------- constants
B, T, C = 4, 2048, 1024
H, D = 16, 64
NCORES = 8
HGROUPS = NCORES // B          # 2 head groups
HLOC = H // HGROUPS            # 8 heads per core
DQ = HLOC * D                  # 512 head-dims per core
P = 128
IC = 512                       # i-chunk (query) width


def _import_concourse():
    try:
        import concourse.bass  # noqa: F401
    except ImportError:
        import sys

        for p in ("/opt/trn_rl_repo", "/root/.axon_site/_ro/trn_rl_repo"):
            if p not in sys.path:
                sys.path.insert(0, p)
        import concourse.bass  # noqa: F401


def build_program(t=T, c=C, hloc=HLOC, d=D):
    """Build the single-core Bass program (the same program runs SPMD on 8)."""
    _import_concourse()
    import concourse.bass as bass
    import concourse.mybir as mybir
    import concourse.tile as tile

    assert c % P == 0 and t % IC == 0 and hloc % 2 == 0 and d == 64
    dq = hloc * d                  # local q/k/v width
    CK = c // P                    # contraction chunks over channels
    TI = t // IC                   # i-chunks
    JPC = IC // P                  # j-blocks per i-chunk (4)
    DCH = dq // P                  # q/k/y dout chunks
    HP = hloc // 2                 # head pairs
    F32 = mybir.dt.float32
    R32 = mybir.dt.float32r
    BF16 = mybir.dt.bfloat16
    EXP = mybir.ActivationFunctionType.Exp
    LN = mybir.ActivationFunctionType.Ln
    SCALE = 1.0 / float(np.sqrt(d))

    nc = bass.Bass()
    xT = nc.declare_dram_parameter("xT", [c, t], BF16, isOutput=False)
    wqk = nc.declare_dram_parameter("wqk", [c, 2 * dq], BF16, isOutput=False)
    wv = nc.declare_dram_parameter("wv", [c, dq], BF16, isOutput=False)
    wpr = nc.declare_dram_parameter("wpr", [dq, c], BF16, isOutput=False)
    bqk = nc.declare_dram_parameter("bqk", [P, 2 * DCH], F32, isOutput=False)
    bv = nc.declare_dram_parameter("bv", [1, dq], BF16, isOutput=False)
    out = nc.declare_dram_parameter("out", [t, c], F32, isOutput=True)

    with tile.TileContext(nc) as tc:
        with (
            # float32r is fp32-width; the low-precision guard is dtype-strict
            nc.allow_low_precision(reason="bf16 matmul inputs, fp32 accum"),
            tc.tile_pool(name="const", bufs=1) as const,
            tc.tile_pool(name="xin", bufs=18) as xin,
            tc.tile_pool(name="qpool", bufs=2) as qpool,
            tc.tile_pool(name="kpool", bufs=TI) as kpool,
            tc.tile_pool(name="vpool", bufs=TI) as vpool,
            tc.tile_pool(name="ypool", bufs=2) as ypool,
            tc.tile_pool(name="ptp", bufs=10) as ptp,
            tc.tile_pool(name="bcp", bufs=2) as bcp,
            tc.tile_pool(name="ytp", bufs=2) as ytp,
            tc.tile_pool(name="ostage", bufs=2) as ostage,
            tc.tile_pool(name="ps_mm", bufs=2, space="PSUM") as ps_mm,
            tc.tile_pool(name="ps_st", bufs=3, space="PSUM") as ps_st,
            tc.tile_pool(name="ps_y", bufs=3, space="PSUM") as ps_y,
        ):
            # ---------------- persistent SBUF state
            wqk_sb = const.tile([P, CK, 2 * dq], BF16)
            wv_sb = const.tile([P, CK, dq], BF16)
            wpr_sb = const.tile([P, DCH, c], BF16)
            mask_sb = const.tile([P, JPC, IC], BF16)
            ones_sb = const.tile([P, P], R32)
            ones_bf = const.tile([P, P], BF16)
            bqk_sb = const.tile([P, 2 * DCH], F32)
            bv_sb = const.tile([1, dq], BF16)

            for cc in range(CK):
                nc.sync.dma_start(out=wqk_sb[:, cc, :], in_=wqk[cc * P:(cc + 1) * P, :])
                nc.sync.dma_start(out=wv_sb[:, cc, :], in_=wv[cc * P:(cc + 1) * P, :])
            for dc in range(DCH):
                nc.sync.dma_start(out=wpr_sb[:, dc, :], in_=wpr[dc * P:(dc + 1) * P, :])
            nc.sync.dma_start(out=bqk_sb, in_=bqk[:, :])
            nc.sync.dma_start(out=bv_sb, in_=bv[:, :])

            # memset can't emit float32r directly (invalid ISA); fill an f32
            # scratch and round it via DVE copies
            ones_f32 = const.tile([P, P], F32)
            nc.vector.memset(ones_f32, 1.0)
            nc.vector.tensor_copy(out=ones_sb, in_=ones_f32)
            nc.vector.tensor_copy(out=ones_bf, in_=ones_f32)
            # multiplicative causal masks for the 4 diagonal j-block
            # positions: pattern p is 1 where i_local >= j_local + 128*p
            for pat in range(JPC):
                nc.gpsimd.memset(mask_sb[:, pat, :], 1.0)
                nc.gpsimd.affine_select(
                    out=mask_sb[:, pat, :],
                    in_=mask_sb[:, pat, :],
                    compare_op=mybir.AluOpType.is_ge,
                    fill=0.0,
                    base=-(pat * P),
                    pattern=[[1, IC]],
                    channel_multiplier=-1,
                )

            q_tiles = {}
            k_tiles = {}
            v_tiles = {}

            def load_x(c4):
                isl = slice(c4 * IC, (c4 + 1) * IC)
                xt = []
                for cc in range(CK):
                    xtile = xin.tile([P, IC], BF16, tag="x")
                    nc.sync.dma_start(out=xtile,
                                      in_=xT[cc * P:(cc + 1) * P, isl])
                    xt.append(xtile)
                return xt

            def qkv_thunks(c4, xt):
                """One thunk per PSUM accumulation group; called interleaved
                with the previous chunk's attention to keep PE dense."""
                q_cur = qpool.tile([P, DCH, IC], BF16, tag="q")
                k_cur = kpool.tile([P, DCH, IC], BF16, tag="k")
                v_cur = vpool.tile([P, JPC, hloc, d + 1], BF16, tag="v")
                q_tiles[c4] = q_cur
                k_tiles[c4] = k_cur
                v_tiles[c4] = v_cur
                thunks = []

                def q_group(oc):
                    ps = ps_mm.tile([P, 512], F32, tag="mm")
                    for cc in range(CK):
                        nc.tensor.matmul(
                            ps[:, :IC],
                            lhsT=wqk_sb[:, cc, oc * P:(oc + 1) * P],
                            rhs=xt[cc], start=(cc == 0), stop=(cc == CK - 1))
                    nc.vector.tensor_scalar_add(q_cur[:, oc, :], ps[:, :IC],
                                                bqk_sb[:, oc:oc + 1])

                def k_group(oc):
                    ps = ps_mm.tile([P, 512], F32, tag="mm")
                    for cc in range(CK):
                        nc.tensor.matmul(
                            ps[:, :IC],
                            lhsT=wqk_sb[:, cc, dq + oc * P:dq + (oc + 1) * P],
                            rhs=xt[cc], start=(cc == 0), stop=(cc == CK - 1))
                    nc.vector.tensor_scalar_add(
                        k_cur[:, oc, :], ps[:, :IC],
                        bqk_sb[:, DCH + oc:DCH + oc + 1])

                def v_group(tbl):
                    ps = ps_mm.tile([P, 512], F32, tag="mm")
                    for cc in range(CK):
                        nc.tensor.matmul(
                            ps[:, :dq],
                            lhsT=xt[cc][:, tbl * P:(tbl + 1) * P],
                            rhs=wv_sb[:, cc, :], start=(cc == 0), stop=False)
                    nc.tensor.matmul(ps[:, :dq], lhsT=ones_bf[0:1, :],
                                     rhs=bv_sb[0:1, :], start=False, stop=True)
                    nc.vector.tensor_copy(
                        out=v_cur[:, tbl, :, 0:d],
                        in_=ps[:, :dq].rearrange("p (h e) -> p h e", h=hloc))
                    # ones column for the softmax-denominator accumulator
                    nc.vector.tensor_copy(
                        out=v_cur[:, tbl, :, d:d + 1],
                        in_=ones_bf[:, 0:hloc][:, :, None])

                for oc in range(DCH):
                    thunks.append(lambda oc=oc: q_group(oc))
                    thunks.append(lambda oc=oc: k_group(oc))
                for tbl in range(JPC):
                    thunks.append(lambda tbl=tbl: v_group(tbl))
                return thunks

            def attention_hp(c4, hp, filler=()):
                filler = list(filler)
                q_cur = q_tiles[c4]
                njb = (c4 + 1) * JPC
                BLK = 3   # j-blocks per S-burst (matches ps_st bufs)
                nblk = (njb + BLK - 1) // BLK
                fill_every = max(1, nblk // len(filler)) if filler else 0
                ya = ps_y.tile([d + 1, IC], F32, tag="y")
                yb = ps_y.tile([d + 1, IC], F32, tag="y")
                blk_i = 0
                for j0 in range(0, njb, BLK):
                    jbs = range(j0, min(j0 + BLK, njb))
                    # burst of S matmuls + exps, then the PV matmuls — the
                    # exp of tile n hides behind the S matmul of tile n+1,
                    # so the PE never micro-idles (HAM stays warm)
                    pts = {}
                    for jb in jbs:
                        kc, jl = jb // JPC, jb % JPC
                        for hi, po in ((0, 0), (1, 64)):
                            st = ps_st.tile([P, IC], F32, tag="st")
                            pt = ptp.tile([P, IC], BF16, tag="pt")
                            pts[(jb, hi)] = pt
                            if jb >= c4 * JPC:
                                # diagonal: cols < w0 fully masked (zeroed
                                # on the idle GpSimd engine), triangular
                                # 128-col block masked on DVE
                                pat = jb - c4 * JPC
                                w0 = pat * P
                                nc.tensor.matmul(
                                    st[:, w0:],
                                    lhsT=k_tiles[kc][po:po + 64, hp,
                                                     jl * P:(jl + 1) * P],
                                    rhs=q_cur[po:po + 64, hp, w0:],
                                    start=True, stop=True)
                                nc.scalar.activation(pt[:, w0:], st[:, w0:],
                                                     EXP, scale=SCALE)
                                if w0:
                                    nc.gpsimd.memset(pt[:, :w0], 0.0)
                                nc.vector.tensor_mul(
                                    pt[:, w0:w0 + P], pt[:, w0:w0 + P],
                                    mask_sb[:, pat, w0:w0 + P])
                            else:
                                nc.tensor.matmul(
                                    st,
                                    lhsT=k_tiles[kc][po:po + 64, hp,
                                                     jl * P:(jl + 1) * P],
                                    rhs=q_cur[po:po + 64, hp, :],
                                    start=True, stop=True)
                                nc.scalar.activation(pt, st, EXP, scale=SCALE)
                    for jb in jbs:
                        for hi, po, yps in ((0, 0, ya), (1, 64, yb)):
                            h = 2 * hp + hi
                            nc.tensor.matmul(
                                yps,
                                lhsT=v_tiles[jb // JPC][:, jb % JPC, h, :],
                                rhs=pts[(jb, hi)],
                                start=(jb == 0), stop=(jb == njb - 1))
                    blk_i += 1
                    if filler and blk_i % fill_every == 0:
                        filler.pop(0)()
                for th in filler:
                    th()
                # normalize: y^T[e, i] = Y^T[e, i] * (1/l[i]); 1/l computed
                # as exp(-ln(l)) on ScalarE (DVE reciprocal on [1,512] is
                # 3.3us), then partition-broadcast via a rank-1 PE matmul.
                y_cur = y_tiles[c4]
                for hi, po, yps in ((0, 0, ya), (1, 64, yb)):
                    lrow = bcp.tile([P, IC], F32, tag="lrow")
                    nc.scalar.activation(lrow[d:d + 1, :], yps[d:d + 1, :], LN)
                    rinv = bcp.tile([P, IC], R32, tag="rinv")
                    nc.scalar.activation(rinv[d:d + 1, :], lrow[d:d + 1, :],
                                         EXP, scale=-1.0)
                    bc = ps_y.tile([d + 1, IC], F32, tag="y")
                    nc.tensor.matmul(
                        bc[0:d, :], lhsT=ones_sb[d:d + 1, 0:d],
                        rhs=rinv[d:d + 1, :], start=True, stop=True)
                    bcs = bcp.tile([P, IC], F32, tag="bcs")
                    nc.vector.tensor_copy(out=bcs[0:d, :], in_=bc[0:d, :])
                    if hi == 0:
                        nc.vector.tensor_mul(y_cur[0:d, hp, :],
                                             yps[0:d, :], bcs[0:d, :])
                    else:
                        yt = ytp.tile([P, IC], BF16, tag="yt")
                        nc.vector.tensor_mul(yt[0:d, :],
                                             yps[0:d, :], bcs[0:d, :])
                        # shift to partitions 64..127 (SBUF->SBUF DMA)
                        nc.sync.dma_start(out=y_cur[64:P, hp, :],
                                          in_=yt[0:d, :])

            def proj_thunks(c4):
                y_cur = y_tiles[c4]

                def grp(tbl, oh):
                    tb = c4 * JPC + tbl
                    ps = ps_mm.tile([P, 512], F32, tag="mm")
                    for dc in range(DCH):
                        nc.tensor.matmul(
                            ps,
                            lhsT=y_cur[:, dc, tbl * P:(tbl + 1) * P],
                            rhs=wpr_sb[:, dc, oh * 512:(oh + 1) * 512],
                            start=(dc == 0), stop=(dc == DCH - 1))
                    ost = ostage.tile([P, 512], F32, tag="ost")
                    nc.vector.tensor_copy(out=ost, in_=ps)
                    nc.sync.dma_start(
                        out=out[tb * P:(tb + 1) * P,
                                oh * 512:(oh + 1) * 512],
                        in_=ost)

                return [lambda tbl=tbl, oh=oh: grp(tbl, oh)
                        for tbl in range(JPC) for oh in range(c // 512)]

            # -------------- software pipeline over i-chunks
            y_tiles = {}
            xt = load_x(0)
            for th in qkv_thunks(0, xt):
                th()
            prev_proj = []
            for c4 in range(TI):
                pend = list(prev_proj)
                if c4 + 1 < TI:
                    xt = load_x(c4 + 1)
                    pend += qkv_thunks(c4 + 1, xt)
                y_tiles[c4] = ypool.tile([P, DCH, IC], BF16, tag="ych",
                                         name=f"ych_{c4}")
                per_hp = (len(pend) + HP - 1) // HP if pend else 0
                for hp in range(HP):
                    attention_hp(c4, hp,
                                 filler=pend[hp * per_hp:(hp + 1) * per_hp])
                prev_proj = proj_thunks(c4)
            for th in prev_proj:
                th()

    _split_multi_waits(nc, mybir)
    return nc


def _split_multi_waits(nc, mybir):
    """The walrus build in this image rejects instructions carrying more than
    one sem wait ("Too many sync wait commands").  Tile's exit drain carries
    several; peel the extras onto same-engine nops placed just before."""
    for f in nc.m.functions:
        for blk in f.blocks:
            changed = False
            out_list = []
            for inst in blk.instructions:
                si = inst.sync_info
                if si is not None and len(si.on_wait) > 1:
                    waits = list(si.on_wait)
                    for j, w in enumerate(waits[1:]):
                        nop = mybir.InstNoOp(
                            name=f"{inst.name}-wsplit-{j}", ins=[], outs=[],
                            sync_info=mybir.SyncInfo(on_update=[], on_wait=[w]))
                        nop.engine = inst.engine
                        try:
                            nc.register_instruction(nop, overwrite=True)
                        except Exception:
                            pass
                        out_list.append(nop)
                    si.on_wait = waits[:1]
                    inst.sync_info = si
                    changed = True
                out_list.append(inst)
            if changed:
                blk.instructions = out_list


# ------------------------------------------------------------------- host
_cache = {}


def _get_program():
    if "nc" not in _cache:
        _cache["nc"] = build_program()
    return _cache["nc"]


def make_in_maps(x, W_attn, b_attn, W_proj, b_proj):
    import ml_dtypes

    bf16 = ml_dtypes.bfloat16
    x = np.asarray(x, np.float32)
    W_attn = np.asarray(W_attn, np.float32)
    b_attn = np.asarray(b_attn, np.float32)
    W_proj = np.asarray(W_proj, np.float32)
    in_maps = []
    for core in range(NCORES):
        b = core // HGROUPS
        g = core % HGROUPS
        hs = g * DQ
        wq = W_attn[:, hs:hs + DQ]
        wk = W_attn[:, C + hs:C + hs + DQ]
        wv = W_attn[:, 2 * C + hs:2 * C + hs + DQ]
        bq = b_attn[hs:hs + DQ]
        bk = b_attn[C + hs:C + hs + DQ]
        bv = b_attn[2 * C + hs:2 * C + hs + DQ]
        in_maps.append({
            "xT": np.ascontiguousarray(x[b].T).astype(bf16),
            "wqk": np.concatenate([wq, wk], axis=1).astype(bf16),
            "wv": np.ascontiguousarray(wv).astype(bf16),
            "wpr": np.ascontiguousarray(W_proj[hs:hs + DQ, :]).astype(bf16),
            "bqk": np.ascontiguousarray(
                np.concatenate([bq, bk]).reshape(2 * (DQ // P), P).T),
            "bv": bv.reshape(1, DQ).astype(bf16),
        })
    return in_maps


def combine_outputs(outs, b_proj):
    b_proj = np.asarray(b_proj, np.float32)
    y = np.empty((B, T, C), np.float32)
    for b in range(B):
        y[b] = outs[HGROUPS * b] + outs[HGROUPS * b + 1]
    y += b_proj[None, None, :]
    return y


def kernel(x, W_attn, b_attn, W_proj, b_proj):
    _import_concourse()
    from concourse.bass_utils import run_bass_kernel_spmd

    nc = _get_program()
    in_maps = make_in_maps(x, W_attn, b_attn, W_proj, b_proj)
    res = run_bass_kernel_spmd(nc, in_maps, core_ids=list(range(NCORES)))
    outs = [res.results[i]["out"] for i in range(NCORES)]
    return combine_outputs(outs, b_proj)
